# revision 32
# baseline (speedup 1.0000x reference)
"""Trainium2 Bass kernel for nn_NeuralAESImplementation.

State-major layout, p = 32j + 8i + k  <->  st[b, i, j, k].
  x_0 = |x - K0|                                  (ACT Abs, bias=-K0)
  round rho = 1..9:
    v[j]  = lhsT1[j]^T x                          (PE, 4 mm -> PSUM)
    h     = relu(v + b0)                          (ACT/DVE/Pool staging -> SBUF f32r)
    s     = sum_j (hi_j + lo_j)^T h[j]            (PE, 8 mm accumulate -> PSUM)
    x     = min(max(s, K), ||s-(3-K)| - 1|)       (3-pass gadget == |g4(s)-K|)
  round 10: s = sum_j (finhi_j + finlo_j)^T h[j]; out = |s - K10|

f32r (e8m11) weights pre-rounded; hi/lo split recovers ~23-bit weights on
MM2. Data parallel over 8 cores; batch tiled by NT=512 (PSUM bank).
"""

import os

import numpy as np

B_TOTAL = 131072
N_CORES = 8
B_CORE = B_TOTAL // N_CORES
NT = 512
SBOX_H = 32

_CACHE = {}
_RUN_KWARGS = {}


def _env(name, default):
    return os.environ.get(name, default)


# ---------------------------------------------------------------- host math
def _round_e8m11(a):
    u = np.ascontiguousarray(a, np.float32).view(np.uint32)
    return ((u.astype(np.uint64) + 0x800) & 0xFFFFF000).astype(np.uint32).view(
        np.float32
    )


def _build_mats(w0, w1):
    lhsT1 = np.zeros((4, 128, 128), np.float32)
    lhsT2mix = np.zeros((4, 128, 128), np.float32)
    lhsT2fin = np.zeros((4, 128, 128), np.float32)
    iu = np.arange(32)
    for j in range(4):
        for i in range(4):
            for k in range(8):
                lhsT1[j][32 * j + 8 * i + k, 32 * i + iu] = w0[:, k]
            ip = (i - j) % 4
            for jp in range(4):
                d = (j - jp) % 4
                for k in range(8):
                    if d <= 2:
                        lhsT2mix[j][32 * i + iu, 32 * jp + 8 * ip + k] = w1[k, :]
                    if d == 0:
                        lhsT2fin[j][32 * i + iu, 32 * jp + 8 * ip + k] = w1[k, :]
    return lhsT1, lhsT2mix, lhsT2fin


def _host_prep(round_keys, w0, b0, w1):
    # K[r][p], p = 32j+8i+k <-> round_keys[r,0,i,j,k]
    K = (
        np.transpose(round_keys[:, 0], (0, 2, 1, 3))
        .reshape(11, 128)
        .astype(np.float32)
    )
    lhsT1, lhsT2mix, lhsT2fin = _build_mats(w0, w1)
    mats = []
    mats.append(_round_e8m11(lhsT1))                    # 0..3   MM1 (12-bit)
    mixhi = _round_e8m11(lhsT2mix)
    mats.append(mixhi)                                  # 4..7   MM2 hi
    mats.append(lhsT2mix - mixhi)                       # 8..11  MM2 lo
    finhi = _round_e8m11(lhsT2fin)
    mats.append(finhi)                                  # 12..15 fin hi
    mats.append(lhsT2fin - finhi)                       # 16..19 fin lo
    wmat = np.concatenate(mats, axis=0)                 # [20,128,128]
    # wvec columns:
    #   0      : -K0           (head Abs bias)
    #   1..9   : K_r - 3       (g1 Abs bias, rounds 1..9)
    #   10     : -K10          (final Abs bias)
    #   11..19 : K_r           (g3 max vec, rounds 1..9)
    #   20     : b0 tiled      (staging relu bias)
    #   21     : -1.0          (u abs bias)
    wvec = np.zeros((128, 22), np.float32)
    wvec[:, 21] = -1.0
    wvec[:, 0] = -K[0]
    for r in range(1, 10):
        wvec[:, r] = K[r] - 3.0
        wvec[:, 10 + r] = K[r]
    wvec[:, 10] = -K[10]
    wvec[:, 20] = np.tile(b0, 4)
    return wmat, wvec


def _fallback_numpy(state, round_keys, xorw, xorb, w0, b0, w1):
    def relu(v):
        return np.maximum(v, 0.0)

    def ark(s, k):
        c0 = xorw[0, 0] * s + xorw[0, 1] * k + xorb[0]
        c1 = xorw[1, 0] * s + xorw[1, 1] * k + xorb[1]
        return relu(c0) + relu(c1)

    def sub_bytes(x):
        h = relu(np.einsum("bijk,hk->bijh", x, w0) + b0)
        return np.einsum("bijh,kh->bijk", h, w1)

    def shift_rows(x):
        return np.stack(
            [np.roll(x[:, :, r, :], -r, axis=1) for r in range(4)], axis=2
        )

    def mix_columns(x):
        s = x + np.roll(x, -1, axis=2) + np.roll(x, -2, axis=2)
        return relu(s) - 2 * relu(s - 1) + 2 * relu(s - 2) - 2 * relu(s - 3)

    st = state.reshape(-1, 4, 4, 8).swapaxes(1, 2)
    st = ark(st, round_keys[0])
    for r in range(1, 10):
        st = mix_columns(shift_rows(sub_bytes(st)))
        st = ark(st, round_keys[r])
    st = shift_rows(sub_bytes(st))
    st = ark(st, round_keys[10])
    return np.ascontiguousarray(st.swapaxes(1, 2).reshape(-1, 128), dtype=np.float32)


# ---------------------------------------------------------------- bass program
def _build_bass(b_core):
    import concourse.bacc as bacc
    import concourse.mybir as mybir
    import concourse.tile as tile
    from contextlib import ExitStack

    f32 = mybir.dt.float32
    f32r = mybir.dt.float32r
    f16 = mybir.dt.float16
    alu = mybir.AluOpType
    AF = mybir.ActivationFunctionType
    nchunk = b_core // NT

    # Engine assignment knobs:
    #   staging split: "a2dp" = ACT[1024] + DVE[512] + Pool[512]
    #                  "a2d2" = ACT[1024] + DVE[1024]
    #                  "a3d1" = ACT[1536] + DVE[512]
    stg = _env("NAES_STG", "vring")
    lo_mode = _env("NAES_LO", "0")  # 2=full hi+lo, 1=lo alternate rounds, 0=hi only
    g1_eng = _env("NAES_G1", "act")    # act | dve
    g2_eng = _env("NAES_G2", "alt")    # act | pool | alt (alternate by slot)
    g3_eng = _env("NAES_G3", "dve")    # dve | pool
    gmode = _env("NAES_GMODE", "direct")  # direct | evac
    u16 = _env("NAES_U16", "1") == "1"  # t/u intermediates in fp16
    xdt_s = _env("NAES_XDT", "f32r")   # f32r | f16 (x storage / MM1 rhs)
    nstream = min(int(_env("NAES_NSTREAM", "8")), nchunk)
    assert nchunk % nstream == 0
    lookahead = int(_env("NAES_LA", "1"))

    xdt = f16 if xdt_s == "f16" else f32

    nc = bacc.Bacc()
    st_d = nc.dram_tensor("state", [128, b_core], f32, kind="ExternalInput")
    wm_d = nc.dram_tensor("wmat", [20, 128, 128], f32, kind="ExternalInput")
    wv_d = nc.dram_tensor("wvec", [128, 22], f32, kind="ExternalInput")
    out_d = nc.dram_tensor("out", [128, b_core], f32, kind="ExternalOutput")

    with tile.TileContext(nc) as tc, ExitStack() as ctx:
        wpool = ctx.enter_context(tc.tile_pool(name="weights", bufs=1))
        iopool = ctx.enter_context(tc.tile_pool(name="io", bufs=4))
        xpool = ctx.enter_context(
            tc.tile_pool(name="x", bufs=int(_env("NAES_XB", str(nstream + 2))))
        )
        hpool = ctx.enter_context(tc.tile_pool(name="h", bufs=4))
        gpool = ctx.enter_context(tc.tile_pool(name="gad", bufs=8))
        fpool = ctx.enter_context(tc.tile_pool(name="f", bufs=3))
        ps_v0 = ctx.enter_context(
            tc.tile_pool(name="pv0", bufs=int(_env("NAES_PV0B", "2")), space="PSUM")
        )
        ps_v1 = ctx.enter_context(
            tc.tile_pool(name="pv1", bufs=int(_env("NAES_PV1B", "1")), space="PSUM")
        )
        ps_s = ctx.enter_context(
            tc.tile_pool(name="ps", bufs=int(_env("NAES_PSB", "4")), space="PSUM")
        )

        wsb = wpool.tile([128, 20 * 128], f32, tag="wsb")
        nc.sync.dma_start(
            wsb[:].rearrange("p (m q) -> p m q", m=20).bitcast(f32r),
            wm_d[:].rearrange("m p q -> p m q").bitcast(f32r),
        )
        vec_sb = wpool.tile([128, 22], f32, tag="vec")
        nc.sync.dma_start(vec_sb[:], wv_d[:])

        def W(m):
            return wsb[:, 128 * m : 128 * (m + 1)].bitcast(f32r)

        def emit_head(c, st):
            in_sb = iopool.tile([128, NT], f32, tag="in")
            nc.sync.dma_start(in_sb[:], st_d[:, c * NT : (c + 1) * NT])
            x_sb = xpool.tile([128, NT], xdt, tag="x")
            xout = x_sb[:].bitcast(f32r) if xdt is f32 else x_sb[:]
            nc.scalar.activation(xout, in_sb[:], AF.Abs, bias=vec_sb[:, 0:1])
            st["x"] = x_sb

        def emit_front(c, rho, st):
            """MM1 + h staging for slot (rho, c)."""
            x_sb = st["x"]
            h_sb = hpool.tile([128, 4 * NT], f32, tag="hs")
            b0v = vec_sb[:, 20:21]

            # ---- MM1: v[j] = lhsT1[j]^T x, two PSUM tiles [128, 2*NT].
            if stg == "vring":
                v0t = ps_v0.tile([128, 2 * NT], f32, tag="v")
                v1t = ps_v0.tile([128, 2 * NT], f32, tag="v")
                order = (0, 1, 2, 3)
            else:
                v0t = ps_v0.tile([128, 2 * NT], f32, tag="v0")
                v1t = ps_v1.tile([128, 2 * NT], f32, tag="v1")
                order = (2, 3, 0, 1)
            x_rhs = x_sb[:].bitcast(f32r) if xdt is f32 else x_sb[:]
            for j in order:
                vt = v0t if j < 2 else v1t
                nc.tensor.matmul(
                    vt[:, NT * (j % 2) : NT * (j % 2 + 1)],
                    W(j),
                    x_rhs,
                    start=True,
                    stop=True,
                    skip_group_check=True,
                )
            v0 = v0t[:]
            v1 = v1t[:]

            # ---- staging: h = relu(v + b0) -> SBUF f32r
            def stage_act(src, dst):
                nc.scalar.activation(dst.bitcast(f32r), src, AF.Relu, bias=b0v)

            def stage_dve(src, dst):
                nc.vector.tensor_scalar(
                    dst.bitcast(f32r), src, b0v, 0.0, alu.add, alu.max
                )

            def stage_pool(src, dst):
                nc.gpsimd.tensor_scalar(
                    dst.bitcast(f32r), src, b0v, 0.0, alu.add, alu.max
                )

            if stg == "vring":
                # v-even -> ACT [1024], v-odd -> DVE [1024]; ring of 3.
                stage_act(v0, h_sb[:, 0 : 2 * NT])
                stage_dve(v1, h_sb[:, 2 * NT : 4 * NT])
            elif stg == "advp":
                # v1 (tight ring) staged by the fast engines; v0 (bufs=2,
                # relaxed deadline) by Pool.
                stage_act(v1[:, 0:NT], h_sb[:, 2 * NT : 3 * NT])
                stage_dve(v1[:, NT : 2 * NT], h_sb[:, 3 * NT : 4 * NT])
                stage_pool(v0, h_sb[:, 0 : 2 * NT])
            elif stg == "a2dp":
                stage_act(v0, h_sb[:, 0 : 2 * NT])
                stage_dve(v1[:, 0:NT], h_sb[:, 2 * NT : 3 * NT])
                stage_pool(v1[:, NT : 2 * NT], h_sb[:, 3 * NT : 4 * NT])
            elif stg == "a2d2":
                stage_act(v0, h_sb[:, 0 : 2 * NT])
                stage_dve(v1, h_sb[:, 2 * NT : 4 * NT])
            elif stg == "a3d1":
                stage_act(v0, h_sb[:, 0 : 2 * NT])
                stage_act(v1[:, 0:NT], h_sb[:, 2 * NT : 3 * NT])
                stage_dve(v1[:, NT : 2 * NT], h_sb[:, 3 * NT : 4 * NT])
            else:
                raise ValueError(stg)
            st["h"] = h_sb

        def emit_back(c, rho, st):
            """MM2 + gadget for slot (rho, c)."""
            h_sb = st["h"]
            # ---- MM2: s = sum_j (hi[+lo])[j]^T h[j]
            s_ps = ps_s.tile([128, NT], f32, tag="s")
            base = 4 if rho < 10 else 12
            nlo = 2 if (lo_mode == "2" or (lo_mode == "1" and rho % 2 == 1)) else 1
            n = 0
            for lo in range(nlo):
                for j in range(4):
                    n += 1
                    nc.tensor.matmul(
                        s_ps[:],
                        W(base + 4 * lo + j),
                        h_sb[:, NT * j : NT * (j + 1)].bitcast(f32r),
                        start=(n == 1),
                        stop=(n == 4 * nlo),
                        skip_group_check=True,
                    )

            if rho < 10:
                # ---- gadget: x = min(max(s, K), ||s-(3-K)| - 1|)
                tdt = f16 if u16 else f32
                kv = vec_sb[:, 10 + rho : 11 + rho]
                c1v = vec_sb[:, rho : rho + 1]
                x_sb = xpool.tile([128, NT], xdt, tag="x")
                xout = x_sb[:].bitcast(f32r) if xdt is f32 else x_sb[:]
                # All abs ops must be ACT (abs_max is illegal on DVE/Pool);
                # u = |t-1| alternates: ACT Abs on even slots, Pool
                # negate-pair + DVE tt max on odd slots (load balance).
                t_sb = gpool.tile([128, NT], tdt, tag="gt")
                nc.scalar.activation(t_sb[:], s_ps[:], AF.Abs, bias=c1v)
                u_sb = gpool.tile([128, NT], tdt, tag="gu")
                ualt = (c % int(_env("NAES_UALT", "2")) != 0) if g2_eng == "alt" else (g2_eng == "pool")
                if ualt:
                    tm_sb = gpool.tile([128, NT], tdt, tag="gtm")
                    nc.gpsimd.tensor_scalar(
                        tm_sb[:], t_sb[:], -1.0, 1.0, alu.mult, alu.add
                    )
                    t1_sb = gpool.tile([128, NT], tdt, tag="gt1")
                    nc.gpsimd.tensor_scalar(
                        t1_sb[:], t_sb[:], 1.0, -1.0, alu.mult, alu.add
                    )
                    nc.vector.tensor_tensor(
                        u_sb[:], t1_sb[:], tm_sb[:], alu.max
                    )
                else:
                    nc.scalar.activation(
                        u_sb[:], t_sb[:], AF.Abs, bias=vec_sb[:, 21:22]
                    )
                nc.vector.scalar_tensor_tensor(
                    xout, s_ps[:], kv, u_sb[:], alu.max, alu.min
                )
                st["x"] = x_sb
            else:
                f_sb = fpool.tile([128, NT], f32, tag="fout")
                nc.scalar.activation(
                    f_sb[:], s_ps[:], AF.Abs, bias=vec_sb[:, 10:11]
                )
                nc.sync.dma_start(out_d[:, c * NT : (c + 1) * NT], f_sb[:])

        for grp in range(nchunk // nstream):
            chunks = [grp * nstream + k for k in range(nstream)]
            states = [dict() for _ in chunks]
            for c, s in zip(chunks, states):
                emit_head(c, s)
            # Software-pipelined slots: front(t) runs `lookahead` slots ahead
            # of back(t) so PE never waits on h staging.
            slots = [(rho, k) for rho in range(1, 11) for k in range(nstream)]
            for t, (rho, k) in enumerate(slots):
                emit_front(chunks[k], rho, states[k])
                if t >= lookahead:
                    rho_b, k_b = slots[t - lookahead]
                    emit_back(chunks[k_b], rho_b, states[k_b])
            for t in range(len(slots) - lookahead, len(slots)):
                rho_b, k_b = slots[t]
                emit_back(chunks[k_b], rho_b, states[k_b])

    nc.compile()
    return nc


def _get_bass(b_core):
    key = ("nc", b_core, os.environ.get("NAES_CFG", ""))
    if key not in _CACHE:
        _CACHE[key] = _build_bass(b_core)
    return _CACHE[key]


# ---------------------------------------------------------------- entry point
def kernel(**inputs):
    state = np.ascontiguousarray(np.asarray(inputs["state_tensor"], np.float32))
    rk = np.asarray(inputs["round_keys"], np.float32)
    xorw = np.asarray(inputs["xorw"], np.float32)
    xorb = np.asarray(inputs["xorb"], np.float32)
    w0 = np.asarray(inputs["sbox_w0"], np.float32)
    b0 = np.asarray(inputs["sbox_b0"], np.float32)
    w1 = np.asarray(inputs["sbox_w1"], np.float32)

    canonical = (
        np.array_equal(xorw, np.array([[1.0, -1.0], [-1.0, 1.0]], np.float32))
        and np.all(xorb == 0.0)
        and np.all((rk == 0.0) | (rk == 1.0))
        and state.shape == (B_TOTAL, 128)
        and w0.shape == (SBOX_H, 8)
        and w1.shape == (8, SBOX_H)
    )
    if not canonical:
        return _fallback_numpy(state, rk, xorw, xorb, w0, b0, w1)

    wmat, wvec = _host_prep(rk, w0, b0, w1)
    state_t = np.ascontiguousarray(state.T)  # [128, B]

    nc = _get_bass(B_CORE)
    from concourse.bass_utils import run_bass_kernel_spmd

    in_maps = []
    for c in range(N_CORES):
        in_maps.append(
            {
                "state": np.ascontiguousarray(
                    state_t[:, c * B_CORE : (c + 1) * B_CORE]
                ),
                "wmat": wmat,
                "wvec": wvec,
            }
        )
    res = run_bass_kernel_spmd(nc, in_maps, list(range(N_CORES)), **_RUN_KWARGS)
    _CACHE["last_result"] = res
    out_t = np.concatenate([res.results[c]["out"] for c in range(N_CORES)], axis=1)
    return np.ascontiguousarray(out_t.T, np.float32)
